# revision 30
# baseline (speedup 1.0000x reference)
"""Trainium2 Bass kernel for per-pixel bucketed 3x3 conv — dense-packed col-tiled version.

out[b,o,h,w] = sum_p patches[b,p,h,w] * W[buckets[b,h,w], o, p] + bias
  B=4, Cin=8, Cout=8, K=3, H=W=256, NUM_TYPES=216.

Strategy (8 NeuronCores, data-parallel over H, filter table replicated):
  - Each core owns 32 H-rows chosen by a balancing heuristic (minimizes the
    per-type max count over cores, which sets the common layout width).
    Host lays out im2col patches
    feature-major and TYPE-SORTED with a *dense, data-dependent* packing:
    type widths = max pixel count over the 8 cores (the BIR is compiled per
    counts signature, shared SPMD across cores), so there is no fixed-slot
    padding. 216 types are bin-packed into chunks of <=4 types and <=512
    columns; 4 chunks form a superblock sharing one PSUM bank.
  - Device: chunk c of a superblock runs a col-tiled matmul with an [80, 32]
    stationary at tile_position (0, 32c): rows = 72 features + ones row (bias)
    + 7 zero rows (80 = 16-multiple so the DMA spreads over all 16 SDMA
    engines); stationary cols = up to 4 types x 8 Cout, packed densely (a
    chunk's 32-col stationary overlaps the next chunk's weight columns; the
    overlap only produces garbage stripe rows the host ignores).
    The four matmuls write disjoint 32-partition stripes of one [128, W] PSUM
    bank and execute concurrently on the PE (separate column strips/XBUSes).
  - One full-width [128, W] PSUM->SBUF bf16 copy per superblock (ScalarE/DVE
    alternating), staged into a single SBUF buffer; out-DMA is batched over
    superblock groups for large descriptors. Host un-permutes the output.
"""

import numpy as np

B, Cin, Cout, K, H, W = 4, 8, 8, 3, 256, 256
NUM_TYPES = 216
NCORES = 8
RH = H // NCORES          # 32 rows of H per core
NPX = RH * W * B          # pixels per core = 32768
JDIM = 80                 # 72 features + ones row + 7 zero pad (16-multiple for DMA engine spread)
CHMAX = 512               # max chunk width (one PSUM bank of fp32)

_COMPILED = {}


def _balance_rows(buckets):
    """Assign the 256 H-rows to 8 cores (32 each) minimizing sum_t max_core count.

    Greedy: place spikiest rows first on the core where they least increase
    the per-type max envelope. Cuts the cross-core max padding vs contiguous
    H-sharding (the layout width is max over cores per type).
    """
    rowcnt = np.zeros((H, NUM_TYPES), dtype=np.int64)
    for r in range(H):
        rowcnt[r] = np.bincount(buckets[:, r, :].reshape(-1), minlength=NUM_TYPES)
    order = np.argsort(-rowcnt.max(axis=1), kind="stable")
    S = np.zeros((NCORES, NUM_TYPES), dtype=np.int64)
    groups = [[] for _ in range(NCORES)]
    for r in order:
        curmax = S.max(axis=0)
        best, bc = None, None
        for c in range(NCORES):
            if len(groups[c]) == RH:
                continue
            delta = int(np.maximum(S[c] + rowcnt[r] - curmax, 0).sum())
            if best is None or delta < best:
                best, bc = delta, c
        groups[bc].append(int(r))
        S[bc] += rowcnt[r]
    # swap-refinement: pairwise row swaps that lower the max envelope
    improved, sweeps = True, 0
    while improved and sweeps < 40:
        improved = False
        sweeps += 1
        for a in range(NCORES):
            for b in range(a + 1, NCORES):
                mask = np.ones(NCORES, bool)
                mask[[a, b]] = False
                M = S[mask].max(0)
                cur = np.maximum(M, np.maximum(S[a], S[b])).sum()
                ca, cb = rowcnt[groups[a]], rowcnt[groups[b]]
                best = (0, None)
                for i in range(RH):
                    Sa2 = S[a] - ca[i] + cb
                    Sb2 = S[b] - cb + ca[i]
                    newobj = np.maximum(M, np.maximum(Sa2, Sb2)).sum(1)
                    j = int(newobj.argmin())
                    d = int(newobj[j] - cur)
                    if d < best[0]:
                        best = (d, (i, j))
                if best[1]:
                    i, j = best[1]
                    S[a] += cb[j] - ca[i]
                    S[b] += ca[i] - cb[j]
                    groups[a][i], groups[b][j] = groups[b][j], groups[a][i]
                    improved = True
    return [np.asarray(sorted(g)) for g in groups]


def _plan_layout(common):
    """Bin-pack types into chunks (<=4 types, <=CHMAX cols each).

    common: [NUM_TYPES] per-type column width (max count over cores).
    Returns dict with chunk assignment and all derived device/host layout.
    """
    active = [t for t in range(NUM_TYPES) if common[t] > 0]
    order = sorted(active, key=lambda t: -common[t])
    total = int(sum(common[t] for t in active))
    nbins = max((total + CHMAX - 1) // CHMAX, (len(active) + 3) // 4)
    while True:
        nbins_r = ((nbins + 3) // 4) * 4
        bins = [[0, []] for _ in range(nbins_r)]
        ok = True
        for t in order:
            c = int(common[t])
            cand = [b for b in bins if len(b[1]) < 4 and b[0] + c <= CHMAX]
            if not cand:
                ok = False
                break
            b = min(cand, key=lambda b: b[0])
            b[0] += c
            b[1].append(t)
        if ok:
            break
        nbins = nbins_r + 1
    # sort chunks by width desc; group consecutive 4 into superblocks
    bins.sort(key=lambda b: -b[0])
    nch = len(bins)
    nsb = nch // 4
    wmax = [bins[4 * s][0] for s in range(nsb)]          # SB chunk width
    sboff = np.concatenate([[0], np.cumsum([4 * w for w in wmax])])  # pat cols
    ooff = np.concatenate([[0], np.cumsum(wmax)])        # out cols
    NS = int(sboff[-1])
    OW = int(ooff[-1])

    # per-type: chunk id, type slot j, col offset within chunk
    t_chunk = np.full(NUM_TYPES, -1, dtype=np.int64)
    t_slot = np.zeros(NUM_TYPES, dtype=np.int64)
    t_off = np.zeros(NUM_TYPES, dtype=np.int64)
    for g, (_, ts) in enumerate(bins):
        off = 0
        for j, t in enumerate(ts):
            t_chunk[t] = g
            t_slot[t] = j
            t_off[t] = off
            off += int(common[t])

    # dense stationary layout: chunk g's stationary starts at col 8*cumnt[g]
    # and spans 32 cols, overlapping the next chunks' weight columns as pad
    # (pad cols only produce garbage stripe rows the host ignores).
    nts = [len(ts) for _, ts in bins]
    cumnt = np.concatenate([[0], np.cumsum(nts)])
    woff = [int(8 * cumnt[g]) for g in range(nch)]
    wtw = int(8 * cumnt[-1]) + 32   # +32 zero tail for the last chunks' overrun

    g = t_chunk
    s = g // 4
    c = g % 4
    # pat column base of each type (within its chunk region)
    wmax_arr = np.asarray(wmax, dtype=np.int64)
    patbase = sboff[s] + c * wmax_arr[s] + t_off
    # output (partition, col) base of each type
    pbase = 32 * c + 8 * t_slot
    cbase = ooff[s] + t_off

    def taper_splits(n, weights):
        tot = sum(weights)
        acc, cuts = 0.0, [0]
        for w in weights[:-1]:
            acc += w
            cuts.append(max(cuts[-1] + 1, min(n - (len(weights) - len(cuts)), round(n * acc / tot))))
        cuts.append(n)
        return sorted(set(c for c in cuts if 0 <= c <= n))

    psb = taper_splits(nsb, [5, 4, 4, 3, 2, 1])
    bounds = [int(sboff[i]) for i in psb]
    ob = taper_splits(nsb, [5, 4, 3, 3, 2, 1, 1])
    obounds = [int(ooff[i]) for i in ob]

    return {
        "bins": bins, "nch": nch, "nsb": nsb, "wmax": wmax,
        "sboff": sboff, "ooff": ooff, "NS": NS, "OW": OW,
        "patbase": patbase, "pbase": pbase, "cbase": cbase,
        "bounds": bounds, "ob": ob, "obounds": obounds,
        "woff": woff, "wtw": wtw,
    }


def _build_nc(lay):
    from concourse import bacc, mybir
    from concourse.tile import TileContext

    nsb, wmax = lay["nsb"], lay["wmax"]
    sboff, ooff = lay["sboff"], lay["ooff"]
    NS, OW, nch = lay["NS"], lay["OW"], lay["nch"]

    nc = bacc.Bacc(None, target_bir_lowering=False, debug=False)
    bf16 = mybir.dt.bfloat16

    # pat DMA pieces split at SB boundaries, tapering toward the end so the
    # matmul/copy/out chase after the last piece is short.
    bounds, ob = lay["bounds"], lay["ob"]
    pat_ext = nc.declare_dram_parameter("pat", [JDIM, NS], bf16, isOutput=False)
    wt_ext = nc.declare_dram_parameter("wt", [JDIM, lay["wtw"]], bf16, isOutput=False)
    out_ext = nc.declare_dram_parameter("out", [128, OW], bf16, isOutput=True)

    with TileContext(nc) as tc:
        with (
            tc.tile_pool(name="main", bufs=1) as mpool,
            tc.tile_pool(name="ps", bufs=6, space="PSUM") as pspool,
        ):
            wt_sb = mpool.tile([JDIM, lay["wtw"]], bf16)
            nc.scalar.dma_start(out=wt_sb[:], in_=wt_ext[:, :])
            pat_sb = mpool.tile([JDIM, NS], bf16)
            for i in range(len(bounds) - 1):
                c0, c1 = bounds[i], bounds[i + 1]
                q = nc.sync if i % 2 == 0 else nc.scalar
                q.dma_start(out=pat_sb[:, c0:c1], in_=pat_ext[:, c0:c1])

            stg = mpool.tile([128, OW], bf16)
            obi = 1
            for s in range(nsb):
                w = wmax[s]
                ps = pspool.tile([128, w], mybir.dt.float32, tag="ps")
                for c in range(4):
                    g = 4 * s + c
                    k0 = int(sboff[s]) + c * w
                    nc.tensor.matmul(
                        out=ps[32 * c : 32 * c + 32, :],
                        lhsT=wt_sb[:, lay["woff"][g] : lay["woff"][g] + 32],
                        rhs=pat_sb[:, k0 : k0 + w],
                        start=True,
                        stop=True,
                        tile_position=(0, 32 * c),
                    )
                o0 = int(ooff[s])
                # split each SB's PSUM->SBUF copy across both engines, sized
                # to their throughputs (ACT ~0.96 vs DVE ~1.37 elem/cyc) so
                # both halves finish together (tightens the tail chase)
                h = (int(w * 0.41)) & ~1
                nc.scalar.activation(
                    out=stg[:, o0 : o0 + h],
                    in_=ps[:, 0:h],
                    func=mybir.ActivationFunctionType.Copy,
                )
                nc.vector.tensor_scalar_mul(
                    stg[:, o0 + h : o0 + w], ps[:, h:w], 1.0
                )
                if obi < len(ob) and s == ob[obi] - 1:
                    a, b = int(ooff[ob[obi - 1]]), int(ooff[ob[obi]])
                    nc.sync.dma_start(out=out_ext[:, a:b], in_=stg[:, a:b])
                    obi += 1
    nc.compile()
    return nc


def _prep_inputs(x, filter_emb, buckets):
    """Host-side layout prep. Returns (nc, in_maps, per-core (part, col), core_rows)."""
    import ml_dtypes

    bf16 = ml_dtypes.bfloat16
    x = np.asarray(x, dtype=np.float32)
    filter_emb = np.asarray(filter_emb, dtype=np.float32)
    buckets = np.asarray(buckets).astype(np.int64)

    # balanced row->core assignment, per-core counts, cross-core max layout
    core_rows = _balance_rows(buckets)
    tcores = []
    counts = np.zeros((NCORES, NUM_TYPES), dtype=np.int64)
    for ci in range(NCORES):
        tcore = buckets[:, core_rows[ci], :].reshape(NPX)
        tcores.append(tcore)
        counts[ci] = np.bincount(tcore, minlength=NUM_TYPES)
    common = counts.max(axis=0)
    assert common.max() <= CHMAX, common.max()

    key = common.tobytes()
    if key in _COMPILED:
        lay, nc = _COMPILED[key]
    else:
        lay = _plan_layout(common)
        nc = _build_nc(lay)
        _COMPILED.clear()
        _COMPILED[key] = (lay, nc)

    # --- weight stationary [JDIM, nch*32] ---
    nw = Cout * Cin * K * K
    wmat = filter_emb[:, :nw].reshape(NUM_TYPES, Cout, Cin * K * K)
    bias = filter_emb[:, nw:]                      # [216, 8]
    w72 = wmat.transpose(2, 0, 1)                  # [72, 216, 8]
    wt = np.zeros((JDIM, lay["wtw"]), dtype=np.float32)
    for g, (_, ts) in enumerate(lay["bins"]):
        for j, t in enumerate(ts):
            o = lay["woff"][g] + 8 * j
            wt[:72, o : o + 8] = w72[:, t, :]
            wt[72, o : o + 8] = bias[t]
    wt = wt.astype(bf16)

    # --- im2col patches, feature order (c, kh, kw) ---
    xp = np.pad(x, ((0, 0), (0, 0), (1, 1), (1, 1)))
    sw = np.lib.stride_tricks.sliding_window_view(xp, (K, K), axis=(2, 3))
    patches = sw.transpose(0, 2, 3, 1, 4, 5).reshape(B, H, W, Cin * K * K)

    patbase = lay["patbase"]
    pbase, cbase = lay["pbase"], lay["cbase"]
    NS = lay["NS"]

    in_maps = []
    unperm = []
    for ci in range(NCORES):
        tcore = tcores[ci]
        cnt = counts[ci]
        order = np.argsort(tcore, kind="stable")
        starts = np.zeros(NUM_TYPES, dtype=np.int64)
        starts[1:] = np.cumsum(cnt)[:-1]
        rank = np.arange(NPX) - starts[tcore[order]]
        slot = np.empty(NPX, dtype=np.int64)
        slot[order] = patbase[tcore[order]] + rank   # pat col of each pixel

        pslab = patches[:, core_rows[ci]].reshape(NPX, 72)
        patT = np.zeros((NS, JDIM), dtype=np.float32)
        patT[slot, :72] = pslab
        patT[slot, 72] = 1.0
        patT = np.ascontiguousarray(patT.T).astype(bf16)

        part = np.empty(NPX, dtype=np.int64)
        col = np.empty(NPX, dtype=np.int64)
        part[order] = pbase[tcore[order]]
        col[order] = cbase[tcore[order]] + rank
        unperm.append((part, col))

        in_maps.append({"pat": patT, "wt": wt})
    return nc, in_maps, unperm, core_rows


def kernel(x, filter_emb, buckets):
    from concourse.bass_utils import run_bass_kernel_spmd

    nc, in_maps, unperm, core_rows = _prep_inputs(x, filter_emb, buckets)
    res = run_bass_kernel_spmd(nc, in_maps, core_ids=list(range(NCORES)))

    out = np.empty((B, Cout, H, W), dtype=np.float32)
    oidx = np.arange(Cout)
    for ci in range(NCORES):
        o = np.asarray(res.results[ci]["out"]).astype(np.float32)  # [128, OW]
        part, col = unperm[ci]
        opix = o[part[:, None] + oidx[None, :], col[:, None]]      # [NPX, 8]
        out[:, :, core_rows[ci], :] = (
            opix.reshape(B, RH, W, Cout).transpose(0, 3, 1, 2)
        )
    return out
